# revision 1
# baseline (speedup 1.0000x reference)
"""CURVGT GNN message-passing kernel for 8 TRN2 NeuronCores.

Strategy: two device passes with window-aligned one-hot matmuls.
  Pass G: edges sharded by src-range (8 cores), sorted by src. x_j gathered
    via per-128-node-window one-hot matmuls (dynamic-AP rhs slices), computes
    parallel transport pt and u = <pt, att[3:6]> per edge.
  Pass S: edges sharded by dst-range, sorted by dst. Gathers g_i = <x_i,
    att[0:3]> via window matmuls, computes segment softmax numerator/
    denominator payloads, scatters them into a PSUM-resident per-node
    accumulator via one-hot matmuls, finalizes out = num/(den + 1e-16).
Host work is limited to sharding/layout: sorting+bucketing edge ids,
slicing/transposing input arrays, and re-ordering the (pt,u) intermediate
between the two passes. All bulk compute, gathers, and reductions run on
device. Exploits k=k2=ones, attn_p=ones (verified at runtime): the
curvature branch reduces to m1=m2=sum(pt)*ones, feats=0, lin=b1 (constant
per node under softmax), as in the spec's input distribution.
"""
import sys, math, time
sys.path.insert(0, "/opt/trn_rl_repo")
import numpy as np

P = 128
V, E, B = 150000, 900000, 2
N = B * V
BE = B * E
NC = 8
NWIN = 293
R = NWIN * P            # 37504 nodes per core
NTILE = 2000            # padded edge-slot tiles per core (256000 slots)
NTG, NTS = 48, 32       # chunk sizes (tiles) for G and S

_CACHE = {}


def _build_programs():
    if "G" in _CACHE:
        return
    import concourse.bacc as bacc
    import concourse.bass as bass
    import concourse.mybir as mybir
    import concourse.tile as tile

    F = mybir.dt.float32
    I32 = mybir.dt.int32
    PE = mybir.EngineType.PE
    AF = mybir.ActivationFunctionType
    ALU = mybir.AluOpType
    AX = mybir.AxisListType

    def build_G(ntile, nwin, nt_chunk):
        nc = bacc.Bacc("TRN2", target_bir_lowering=False, debug=False,
                       num_devices=NC)
        xg_d = nc.dram_tensor("xg", [P, nwin * 4], F, kind="ExternalInput").ap()
        ev_d = nc.dram_tensor("ev18", [P, ntile, 18], F, kind="ExternalInput").ap()
        hyp_d = nc.dram_tensor("hyp", [P, ntile, 4], F, kind="ExternalInput").ap()
        th_d = nc.dram_tensor("th", [P, ntile], F, kind="ExternalInput").ap()
        om_d = nc.dram_tensor("om", [P, ntile], F, kind="ExternalInput").ap()
        srclf_d = nc.dram_tensor("srclf", [1, ntile * P], F, kind="ExternalInput").ap()
        wt4_d = nc.dram_tensor("wt4", [1, ntile], I32, kind="ExternalInput").ap()
        attB_d = nc.dram_tensor("attB", [P, 3], F, kind="ExternalInput").ap()
        ptu_d = nc.dram_tensor("ptu", [P, ntile, 4], F, kind="ExternalOutput").ap()

        nchunk = math.ceil(ntile / nt_chunk)
        with tile.TileContext(nc) as tc:
            with tc.tile_pool(name="cst", bufs=1) as cst, \
                 tc.tile_pool(name="sb", bufs=2) as sb, \
                 tc.tile_pool(name="ps", bufs=2, space="PSUM") as ps:
                xg = cst.tile([P, nwin * 4], F)
                nc.sync.dma_start(out=xg[:], in_=xg_d[:])
                wt4 = cst.tile([1, ntile], I32)
                nc.sync.dma_start(out=wt4[:], in_=wt4_d[:])
                attB = cst.tile([P, 3], F)
                nc.sync.dma_start(out=attB[:], in_=attB_d[:])
                iop_i = cst.tile([P, 1], I32)
                nc.gpsimd.iota(iop_i[:], pattern=[[0, 1]], base=0, channel_multiplier=1)
                iop = cst.tile([P, 1], F)
                nc.vector.tensor_copy(out=iop[:], in_=iop_i[:])
                zl = cst.tile([P, P], F)
                nc.vector.memset(zl[:], 0.0)
                zr = cst.tile([P, 4 * nt_chunk], F)
                nc.vector.memset(zr[:], 0.0)

                for ch in range(nchunk):
                    t0 = ch * nt_chunk
                    nt = min(nt_chunk, ntile - t0)
                    ne = nt * P
                    ev = sb.tile([P, nt_chunk, 18], F, tag="ev")
                    nc.sync.dma_start(out=ev[:, :nt], in_=ev_d[:, t0:t0 + nt])
                    hyp = sb.tile([P, nt_chunk, 4], F, tag="hyp")
                    nc.sync.dma_start(out=hyp[:, :nt], in_=hyp_d[:, t0:t0 + nt])
                    th = sb.tile([P, nt_chunk], F, tag="th")
                    nc.sync.dma_start(out=th[:, :nt], in_=th_d[:, t0:t0 + nt])
                    om = sb.tile([P, nt_chunk], F, tag="om")
                    nc.sync.dma_start(out=om[:, :nt], in_=om_d[:, t0:t0 + nt])
                    srclf = sb.tile([1, nt_chunk * P], F, tag="srclf")
                    nc.sync.dma_start(out=srclf[:, :ne],
                                      in_=srclf_d[:, t0 * P:t0 * P + ne])

                    srclr = sb.tile([P, nt_chunk * P], F, tag="srclr")
                    nc.gpsimd.partition_broadcast(srclr[:, :ne], srclf[:1, :ne])
                    oh = sb.tile([P, nt_chunk * P], F, tag="oh")
                    nc.vector.tensor_tensor(
                        out=oh[:, :ne], in0=iop[:].to_broadcast([P, ne]),
                        in1=srclr[:, :ne], op=ALU.is_equal)
                    ohv = oh[:, :ne].rearrange("k (t e) -> k t e", e=P)

                    xjp = ps.tile([P, nt_chunk * 4], F, tag="xj")
                    nc.tensor.matmul(out=xjp[:, :nt * 4], lhsT=zl[:],
                                     rhs=zr[:, :nt * 4], start=True, stop=False)
                    for t in range(nt):
                        regs = nc.alloc_registers(f"w4g_{ch}_{t}", engines=[PE])
                        nc.reg_load(regs, wt4[0:1, t0 + t:t0 + t + 1])
                        w4 = nc.snap(regs, donate=True, min_val=0,
                                     max_val=(nwin - 1) * 4)
                        nc.tensor.matmul(
                            out=xjp[:, t * 4:(t + 1) * 4], lhsT=ohv[:, t],
                            rhs=xg[:, bass.ds(w4, 4)], start=False, stop=False)
                    nc.tensor.matmul(out=xjp[:, :nt * 4], lhsT=zl[:],
                                     rhs=zr[:, :nt * 4], start=False, stop=True)
                    xj = xjp[:, :nt * 4].rearrange("p (t c) -> p t c", c=4)

                    cs = sb.tile([P, nt_chunk, 2], F, tag="cs")
                    g1 = sb.tile([P, nt_chunk], F, tag="g1")
                    g2 = sb.tile([P, nt_chunk], F, tag="g2")
                    d2 = sb.tile([P, nt_chunk], F, tag="d2")
                    thr = sb.tile([P, nt_chunk], F, tag="thr")
                    nc.vector.tensor_scalar(g1[:, :nt], th[:, :nt], math.pi, None, ALU.is_gt)
                    nc.vector.tensor_scalar(g2[:, :nt], th[:, :nt], -math.pi, None, ALU.is_lt)
                    nc.vector.tensor_tensor(out=d2[:, :nt], in0=g1[:, :nt],
                                            in1=g2[:, :nt], op=ALU.subtract)
                    nc.vector.tensor_scalar(d2[:, :nt], d2[:, :nt], 2 * math.pi, None, ALU.mult)
                    nc.vector.tensor_tensor(out=thr[:, :nt], in0=th[:, :nt],
                                            in1=d2[:, :nt], op=ALU.subtract)
                    nc.scalar.activation(cs[:, :nt, 1], thr[:, :nt], AF.Sin)
                    thc = sb.tile([P, nt_chunk], F, tag="thc")
                    nc.vector.tensor_scalar(thc[:, :nt], th[:, :nt], math.pi / 2, None, ALU.add)
                    nc.vector.tensor_scalar(g1[:, :nt], thc[:, :nt], math.pi, None, ALU.is_gt)
                    nc.vector.tensor_scalar(g2[:, :nt], thc[:, :nt], -math.pi, None, ALU.is_lt)
                    nc.vector.tensor_tensor(out=d2[:, :nt], in0=g1[:, :nt],
                                            in1=g2[:, :nt], op=ALU.subtract)
                    nc.vector.tensor_scalar(d2[:, :nt], d2[:, :nt], 2 * math.pi, None, ALU.mult)
                    nc.vector.tensor_tensor(out=thc[:, :nt], in0=thc[:, :nt],
                                            in1=d2[:, :nt], op=ALU.subtract)
                    nc.scalar.activation(cs[:, :nt, 0], thc[:, :nt], AF.Sin)

                    t6 = sb.tile([P, nt_chunk, 2, 3], F, tag="t6")
                    ab = sb.tile([P, nt_chunk, 2], F, tag="ab")
                    nc.vector.tensor_tensor(
                        out=t6[:, :nt],
                        in0=ev[:, :nt, 0:6].rearrange("p t (v c) -> p t v c", c=3),
                        in1=xj[:, :, 0:3].rearrange("p t (o c) -> p t o c", o=1)
                            .to_broadcast([P, nt, 2, 3]),
                        op=ALU.mult)
                    nc.vector.tensor_reduce(out=ab[:, :nt], in_=t6[:, :nt],
                                            axis=AX.X, op=ALU.add)
                    t6b = sb.tile([P, nt_chunk, 2, 3], F, tag="t6b")
                    ab2 = sb.tile([P, nt_chunk, 2], F, tag="ab2")
                    nc.vector.tensor_tensor(
                        out=t6b[:, :nt],
                        in0=ev[:, :nt, 9:15].rearrange("p t (v c) -> p t v c", c=3),
                        in1=xj[:, :, 0:3].rearrange("p t (o c) -> p t o c", o=1)
                            .to_broadcast([P, nt, 2, 3]),
                        op=ALU.mult)
                    nc.vector.tensor_reduce(out=ab2[:, :nt], in_=t6b[:, :nt],
                                            axis=AX.X, op=ALU.add)
                    t4 = sb.tile([P, nt_chunk, 2, 2], F, tag="t4")
                    lc = sb.tile([P, nt_chunk, 2], F, tag="lc")
                    nc.vector.tensor_tensor(
                        out=t4[:, :nt],
                        in0=hyp[:, :nt].rearrange("p t (v c) -> p t v c", c=2),
                        in1=ab2[:, :nt].rearrange("p t (o c) -> p t o c", o=1)
                            .to_broadcast([P, nt, 2, 2]),
                        op=ALU.mult)
                    nc.vector.tensor_reduce(out=lc[:, :nt], in_=t4[:, :nt],
                                            axis=AX.X, op=ALU.add)

                    m1 = sb.tile([P, nt_chunk], F, tag="m1")
                    nc.vector.tensor_scalar(m1[:, :nt], om[:, :nt], 1.0, None,
                                            ALU.is_equal)
                    mm = sb.tile([P, nt_chunk], F, tag="mm")
                    nc.vector.tensor_scalar(mm[:, :nt], om[:, :nt], -1.0, None,
                                            ALU.is_equal)
                    m0 = sb.tile([P, nt_chunk], F, tag="m0")
                    nc.vector.tensor_scalar(m0[:, :nt], om[:, :nt], 0.0, None,
                                            ALU.is_equal)

                    co = sb.tile([P, nt_chunk, 6], F, tag="co")
                    am1 = sb.tile([P, nt_chunk], F, tag="am1")
                    nc.vector.tensor_tensor(out=am1[:, :nt], in0=ab[:, :nt, 0],
                                            in1=m1[:, :nt], op=ALU.mult)
                    nc.vector.tensor_tensor(
                        out=co[:, :nt, 0:3:2],
                        in0=am1[:, :nt].rearrange("p (t o) -> p t o", o=1)
                            .to_broadcast([P, nt, 2]),
                        in1=cs[:, :nt], op=ALU.mult)
                    nc.vector.tensor_tensor(out=co[:, :nt, 1], in0=ab[:, :nt, 1],
                                            in1=m1[:, :nt], op=ALU.mult)
                    nc.vector.tensor_tensor(
                        out=co[:, :nt, 3:5], in0=lc[:, :nt],
                        in1=mm[:, :nt].rearrange("p (t o) -> p t o", o=1)
                            .to_broadcast([P, nt, 2]),
                        op=ALU.mult)
                    nc.vector.tensor_copy(out=co[:, :nt, 5], in_=m0[:, :nt])
                    nc.vector.tensor_copy(out=ev[:, :nt, 15:18], in_=xj[:, :, 0:3])

                    big = sb.tile([P, nt_chunk, 3, 6], F, tag="big")
                    ptu = sb.tile([P, nt_chunk, 4], F, tag="ptu")
                    nc.vector.tensor_tensor(
                        out=big[:, :nt],
                        in0=co[:, :nt].rearrange("p t (o k) -> p t o k", o=1)
                            .to_broadcast([P, nt, 3, 6]),
                        in1=ev[:, :nt].rearrange("p t (k c) -> p t c k", c=3),
                        op=ALU.mult)
                    nc.vector.tensor_reduce(out=ptu[:, :nt, 0:3], in_=big[:, :nt],
                                            axis=AX.X, op=ALU.add)
                    t3 = sb.tile([P, nt_chunk, 3], F, tag="t3")
                    nc.vector.tensor_tensor(
                        out=t3[:, :nt], in0=ptu[:, :nt, 0:3],
                        in1=attB[:].rearrange("p (o c) -> p o c", o=1)
                            .to_broadcast([P, nt, 3]),
                        op=ALU.mult)
                    nc.vector.tensor_reduce(out=ptu[:, :nt, 3], in_=t3[:, :nt],
                                            axis=AX.X, op=ALU.add)
                    nc.sync.dma_start(out=ptu_d[:, t0:t0 + nt], in_=ptu[:, :nt])
        nc.compile()
        return nc

    def build_S(ntile, nwin, nt_chunk):
        nc = bacc.Bacc("TRN2", target_bir_lowering=False, debug=False,
                       num_devices=NC)
        xt_d = nc.dram_tensor("xt", [P, nwin, 3], F, kind="ExternalInput").ap()
        ptu_d = nc.dram_tensor("ptu", [P, ntile, 4], F, kind="ExternalInput").ap()
        dstl_d = nc.dram_tensor("dstl", [P, ntile], F, kind="ExternalInput").ap()
        dstlf_d = nc.dram_tensor("dstlf", [1, ntile * P], F, kind="ExternalInput").ap()
        wws_d = nc.dram_tensor("wws", [1, ntile * 2], I32, kind="ExternalInput").ap()
        attA_d = nc.dram_tensor("attA", [P, 3], F, kind="ExternalInput").ap()
        kc_d = nc.dram_tensor("kc", [P, 1], F, kind="ExternalInput").ap()
        iotaP_d = nc.dram_tensor("iotaP", [P, P], F, kind="ExternalInput").ap()
        out_d = nc.dram_tensor("outw", [P, nwin, 3], F, kind="ExternalOutput").ap()

        nchunk = math.ceil(ntile / nt_chunk)
        with tile.TileContext(nc) as tc:
            with tc.tile_pool(name="cst", bufs=1) as cst, \
                 tc.tile_pool(name="sb", bufs=2) as sb, \
                 tc.tile_pool(name="ps", bufs=2, space="PSUM") as ps, \
                 tc.tile_pool(name="psa", bufs=1, space="PSUM") as psa:
                wws = cst.tile([1, ntile * 2], I32)
                nc.sync.dma_start(out=wws[:], in_=wws_d[:])
                attA = cst.tile([P, 3], F)
                nc.sync.dma_start(out=attA[:], in_=attA_d[:])
                kc = cst.tile([P, 1], F)
                nc.sync.dma_start(out=kc[:], in_=kc_d[:])
                iotaP = cst.tile([P, P], F)
                nc.sync.dma_start(out=iotaP[:], in_=iotaP_d[:])
                iop_i = cst.tile([P, 1], I32)
                nc.gpsimd.iota(iop_i[:], pattern=[[0, 1]], base=0, channel_multiplier=1)
                iop = cst.tile([P, 1], F)
                nc.vector.tensor_copy(out=iop[:], in_=iop_i[:])
                zl = cst.tile([P, P], F)
                nc.vector.memset(zl[:], 0.0)
                zr = cst.tile([P, 512], F)
                nc.vector.memset(zr[:], 0.0)

                xt = cst.tile([P, nwin, 3], F)
                nc.sync.dma_start(out=xt[:], in_=xt_d[:])
                gm = cst.tile([P, nwin, 3], F)
                nc.vector.tensor_tensor(
                    out=gm[:], in0=xt[:],
                    in1=attA[:].rearrange("p (o c) -> p o c", o=1)
                        .to_broadcast([P, nwin, 3]),
                    op=ALU.mult)
                g2 = cst.tile([P, nwin], F)
                nc.vector.tensor_reduce(out=g2[:], in_=gm[:], axis=AX.X, op=ALU.add)

                acc = psa.tile([P, nwin * 4], F)
                for b0 in range(0, nwin * 4, 512):
                    bn = min(512, nwin * 4 - b0)
                    nc.tensor.matmul(out=acc[:, b0:b0 + bn], lhsT=zl[:],
                                     rhs=zr[:, :bn], start=True, stop=False)

                for ch in range(nchunk):
                    t0 = ch * nt_chunk
                    nt = min(nt_chunk, ntile - t0)
                    ne = nt * P
                    ptu = sb.tile([P, nt_chunk, 4], F, tag="ptu")
                    nc.sync.dma_start(out=ptu[:, :nt], in_=ptu_d[:, t0:t0 + nt])
                    dstl = sb.tile([P, nt_chunk], F, tag="dstl")
                    nc.sync.dma_start(out=dstl[:, :nt], in_=dstl_d[:, t0:t0 + nt])
                    dstlf = sb.tile([1, nt_chunk * P], F, tag="dstlf")
                    nc.sync.dma_start(out=dstlf[:, :ne],
                                      in_=dstlf_d[:, t0 * P:t0 * P + ne])

                    dstlr = sb.tile([P, nt_chunk * P], F, tag="dstlr")
                    nc.gpsimd.partition_broadcast(dstlr[:, :ne], dstlf[:1, :ne])
                    oh = sb.tile([P, nt_chunk * P], F, tag="oh")
                    nc.vector.tensor_tensor(
                        out=oh[:, :ne], in0=iop[:].to_broadcast([P, ne]),
                        in1=dstlr[:, :ne], op=ALU.is_equal)
                    ohv = oh[:, :ne].rearrange("k (t e) -> k t e", e=P)
                    oht = sb.tile([P, nt_chunk * P], F, tag="oht")
                    nc.vector.tensor_tensor(
                        out=oht[:, :ne].rearrange("e (t k) -> e t k", k=P),
                        in0=iotaP[:].rearrange("e (o k) -> e o k", o=1)
                            .to_broadcast([P, nt, P]),
                        in1=dstl[:, :nt].rearrange("e (t o) -> e t o", o=1)
                            .to_broadcast([P, nt, P]),
                        op=ALU.is_equal)
                    ohtv = oht[:, :ne].rearrange("e (t k) -> e t k", k=P)

                    gip = ps.tile([P, nt_chunk], F, tag="gi")
                    nc.tensor.matmul(out=gip[:, :nt], lhsT=zl[:], rhs=zr[:, :nt],
                                     start=True, stop=False)
                    for t in range(nt):
                        regs = nc.alloc_registers(f"wg_{ch}_{t}", engines=[PE])
                        nc.reg_load(regs, wws[0:1, 2 * (t0 + t):2 * (t0 + t) + 1])
                        w = nc.snap(regs, donate=True, min_val=0, max_val=nwin - 1)
                        nc.tensor.matmul(
                            out=gip[:, t:t + 1], lhsT=ohv[:, t],
                            rhs=g2[:, bass.ds(w, 1)], start=False, stop=False)
                    nc.tensor.matmul(out=gip[:, :nt], lhsT=zl[:], rhs=zr[:, :nt],
                                     start=False, stop=True)

                    z = sb.tile([P, nt_chunk], F, tag="z")
                    nc.vector.tensor_tensor(out=z[:, :nt], in0=gip[:, :nt],
                                            in1=ptu[:, :nt, 3], op=ALU.add)
                    z2 = sb.tile([P, nt_chunk], F, tag="z2")
                    nc.vector.tensor_scalar(z2[:, :nt], z[:, :nt], 0.2, None, ALU.mult)
                    gat = sb.tile([P, nt_chunk], F, tag="gat")
                    nc.vector.tensor_tensor(out=gat[:, :nt], in0=z[:, :nt],
                                            in1=z2[:, :nt], op=ALU.max)
                    pay = sb.tile([P, nt_chunk, 4], F, tag="pay")
                    nc.scalar.activation(pay[:, :nt, 0], gat[:, :nt], AF.Exp)

                    s = sb.tile([P, nt_chunk], F, tag="s")
                    nc.vector.tensor_reduce(out=s[:, :nt], in_=ptu[:, :nt, 0:3],
                                            axis=AX.X, op=ALU.add)
                    ks = sb.tile([P, nt_chunk], F, tag="ks")
                    nc.vector.tensor_scalar(ks[:, :nt], s[:, :nt], kc[:, 0:1],
                                            None, ALU.mult)
                    v3 = sb.tile([P, nt_chunk, 3], F, tag="v3")
                    nc.vector.tensor_tensor(
                        out=v3[:, :nt], in0=ptu[:, :nt, 0:3],
                        in1=ks[:, :nt].rearrange("p (t o) -> p t o", o=1)
                            .to_broadcast([P, nt, 3]),
                        op=ALU.add)
                    nc.vector.tensor_tensor(
                        out=pay[:, :nt, 1:4], in0=v3[:, :nt],
                        in1=pay[:, :nt, 0].rearrange("p (t o) -> p t o", o=1)
                            .to_broadcast([P, nt, 3]),
                        op=ALU.mult)

                    for t in range(nt):
                        regs = nc.alloc_registers(f"w4s_{ch}_{t}", engines=[PE])
                        nc.reg_load(regs, wws[0:1, 2 * (t0 + t) + 1:2 * (t0 + t) + 2])
                        w4 = nc.snap(regs, donate=True, min_val=0,
                                     max_val=(nwin - 1) * 4)
                        nc.tensor.matmul(
                            out=acc[:, bass.ds(w4, 4)], lhsT=ohtv[:, t],
                            rhs=pay[:, t], start=False, stop=False)

                for b0 in range(0, nwin * 4, 512):
                    bn = min(512, nwin * 4 - b0)
                    nc.tensor.matmul(out=acc[:, b0:b0 + bn], lhsT=zl[:],
                                     rhs=zr[:, :bn], start=False, stop=True)
                accs = cst.tile([P, nwin, 4], F)
                nc.vector.tensor_copy(out=accs[:],
                                      in_=acc[:].rearrange("p (w c) -> p w c", c=4))
                den = cst.tile([P, nwin], F)
                nc.vector.tensor_scalar(den[:], accs[:, :, 0], 1e-16, None, ALU.add)
                rec = cst.tile([P, nwin], F)
                nc.vector.reciprocal(rec[:], den[:])
                outw = cst.tile([P, nwin, 3], F)
                nc.vector.tensor_tensor(
                    out=outw[:], in0=accs[:, :, 1:4],
                    in1=rec[:].rearrange("p (w o) -> p w o", o=1)
                        .to_broadcast([P, nwin, 3]),
                    op=ALU.mult)
                nc.sync.dma_start(out=out_d[:], in_=outw[:])
        nc.compile()
        return nc

    _CACHE["G"] = build_G(NTILE, NWIN, NTG)
    _CACHE["S"] = build_S(NTILE, NWIN, NTS)


class _Runner:
    def __init__(self, nc):
        import jax
        import jax.numpy  # noqa
        from jax.sharding import Mesh, PartitionSpec, NamedSharding
        from jax.experimental.shard_map import shard_map
        import concourse.mybir as mybir
        from concourse.bass2jax import (_bass_exec_p, install_neuronx_cc_hook,
                                        partition_id_tensor)
        install_neuronx_cc_hook()
        self.jax = jax
        in_names, out_names, out_avals, zero_outs = [], [], [], []
        pname = nc.partition_id_tensor.name if nc.partition_id_tensor else None
        for alloc in nc.m.functions[0].allocations:
            if not isinstance(alloc, mybir.MemoryLocationSet):
                continue
            name = alloc.memorylocations[0].name
            if alloc.kind == "ExternalInput":
                if name != pname:
                    in_names.append(name)
            elif alloc.kind == "ExternalOutput":
                shape = tuple(alloc.tensor_shape)
                dtype = mybir.dt.np(alloc.dtype)
                out_names.append(name)
                out_avals.append(jax.core.ShapedArray(shape, dtype))
                zero_outs.append(np.zeros(shape, dtype))
        self.in_names, self.out_names, self.zero_outs = in_names, out_names, zero_outs
        n_params, n_outs = len(in_names), len(out_names)
        all_names = list(in_names) + list(out_names)
        if pname is not None:
            all_names.append(pname)

        def _body(*args):
            operands = list(args)
            if pname is not None:
                operands.append(partition_id_tensor())
            return tuple(_bass_exec_p.bind(
                *operands, out_avals=tuple(out_avals), in_names=tuple(all_names),
                out_names=tuple(out_names), lowering_input_output_aliases=(),
                sim_require_finite=False, sim_require_nnan=False, nc=nc))

        devices = jax.devices()[:NC]
        mesh = Mesh(np.asarray(devices), ("core",))
        in_specs = (PartitionSpec("core"),) * (n_params + n_outs)
        out_specs = (PartitionSpec("core"),) * n_outs
        self.fn = jax.jit(
            shard_map(_body, mesh=mesh, in_specs=in_specs, out_specs=out_specs,
                      check_rep=False),
            donate_argnums=tuple(range(n_params, n_params + n_outs)),
            keep_unused=True)
        self.sharding = NamedSharding(mesh, PartitionSpec("core"))

    def run(self, in_maps):
        jax = self.jax
        dev_in = [jax.device_put(
            np.concatenate([np.asarray(m[n]) for m in in_maps], axis=0),
            self.sharding) for n in self.in_names]
        dev_out = [jax.device_put(np.concatenate([z] * NC, axis=0), self.sharding)
                   for z in self.zero_outs]
        jax.block_until_ready(dev_in)
        jax.block_until_ready(dev_out)
        t0 = time.perf_counter()
        outs = self.fn(*dev_in, *dev_out)
        jax.block_until_ready(outs)
        dt = time.perf_counter() - t0
        res = [dict() for _ in range(NC)]
        for name, arr in zip(self.out_names, outs):
            arr = np.asarray(arr)
            per = arr.shape[0] // NC
            for c in range(NC):
                res[c][name] = arr[c * per:(c + 1) * per]
        return res, dt


def _slot_layout(arr_slots, ntile, k=None):
    if k is None:
        return np.ascontiguousarray(arr_slots.reshape(ntile, P).T)
    return np.ascontiguousarray(arr_slots.reshape(ntile, P, k).transpose(1, 0, 2))


def _prep_pass(key, ntile, nwin):
    order = np.argsort(key, kind="stable")
    bounds = np.searchsorted(key[order], np.arange(NC + 1) * (nwin * P))
    cores = []
    for c in range(NC):
        idx = order[bounds[c]:bounds[c + 1]]
        loc = key[idx] - c * (nwin * P)
        w = loc >> 7
        cnt = np.bincount(w, minlength=nwin)
        rl = ((cnt + P - 1) // P) * P
        starts = np.concatenate([[0], np.cumsum(rl)]).astype(np.int64)
        assert starts[-1] <= ntile * P, (starts[-1], ntile * P)
        gstart = np.concatenate([[0], np.cumsum(cnt)]).astype(np.int64)
        slot = starts[w] + (np.arange(len(idx)) - gstart[w])
        keyl = np.zeros(ntile * P, np.float32)
        keyl[slot] = (loc & (P - 1)).astype(np.float32)
        tw = np.zeros(ntile, np.int32)
        tws = np.repeat(np.arange(nwin, dtype=np.int32), (rl // P))
        tw[:len(tws)] = tws
        cores.append(dict(idx=idx, slot=slot, keyl=keyl, wt=tw))
    return cores


def _numpy_fallback(inputs):
    def _ln(x, axes):
        mu = x.mean(axis=axes, keepdims=True)
        var = x.var(axis=axes, keepdims=True)
        return (x - mu) / np.sqrt(var + 1e-5)

    x = np.asarray(inputs["x"], np.float32)
    ei = np.asarray(inputs["edge_index"]).astype(np.int64)
    ea = np.asarray(inputs["edge_attrs"], np.float32)
    H2 = np.asarray(inputs["H2frame"], np.float32)
    HPT = np.asarray(inputs["HyperPT"], np.float32)
    omi = np.asarray(inputs["option_mask"]).astype(np.int64)
    bm = np.asarray(inputs["broadcastmap"]).astype(np.int64)
    k = np.asarray(inputs["k"], np.float32); k2 = np.asarray(inputs["k2"], np.float32)
    ap_ = np.asarray(inputs["attn_p"], np.float32)
    att = np.asarray(inputs["att"], np.float32)
    W1 = np.asarray(inputs["W1"], np.float32); b1 = np.asarray(inputs["b1"], np.float32)
    cv = np.asarray(inputs["c"], np.float32)
    src, dst = ei[0], ei[1]

    def tile(a):
        return np.tile(a, (B,) + (1,) * (a.ndim - 1))

    Theta = tile(ea[:, 9:10]); e1 = tile(ea[:, 11:14]); e2 = tile(ea[:, 14:17])
    e3 = tile(ea[:, 17:20]); cos, sin = np.cos(Theta), np.sin(Theta)
    xdir, ydir = tile(H2[:, 0]), tile(H2[:, 1]); T = tile(HPT)
    om = np.tile(omi, B)
    x_j = x[src]; x_i = x[dst]
    a = (e1 * x_j).sum(-1, keepdims=True)
    b = (e2 * x_j).sum(-1, keepdims=True)
    pt1 = a * cos * e1 + a * sin * e3 + b * e2
    a2 = (xdir * x_j).sum(-1, keepdims=True)
    b2 = (ydir * x_j).sum(-1, keepdims=True)
    local = np.concatenate([a2, b2], -1)
    lc2 = np.einsum("eij,ej->ei", T, local)
    pt2 = xdir * lc2[:, 0:1] + ydir * lc2[:, 1:2]
    pt = (pt1 * (om == 1)[:, None] + pt2 * (om == -1)[:, None]
          + x_j * (om == 0)[:, None])
    roots = bm[dst % V]
    m1 = np.einsum("eij,ej->ei", k[roots], pt)
    m2 = np.einsum("eij,ej->ei", k2[roots], pt)
    feats = _ln(np.stack([m1, m2], -1), (1, 2))
    sv = _ln(np.einsum("ecd,edc->ec", ap_[roots], feats), (1,))
    z = np.concatenate([x_i, pt], -1) @ att[0]
    gat = np.where(z > 0, z, 0.2 * z)
    lin = (sv @ W1.T + b1)[:, 0]
    score = gat + lin
    smax = np.full(N, -np.inf, np.float32)
    np.maximum.at(smax, dst, score)
    exps = np.exp(score - smax[dst])
    denom = np.zeros(N, np.float32)
    np.add.at(denom, dst, exps)
    alpha = exps / (denom[dst] + 1e-16)
    msg = alpha[:, None] * (pt + cv[0] * m1 + cv[1] * m2)
    out = np.zeros((N, 3), np.float32)
    np.add.at(out, dst, msg)
    return out


def kernel(**inputs):
    # simplification requires ones-filled curvature tensors (per spec fill)
    ok = (np.all(np.asarray(inputs["k"]) == 1.0)
          and np.all(np.asarray(inputs["k2"]) == 1.0)
          and np.all(np.asarray(inputs["attn_p"]) == 1.0))
    if not ok:
        return _numpy_fallback(inputs)

    ei = np.asarray(inputs["edge_index"]).astype(np.int64)
    src, dst = ei[0], ei[1]
    erow = np.arange(BE) % E
    ea = np.asarray(inputs["edge_attrs"], np.float32)
    ev15_E = np.concatenate(
        [ea[:, 11:20], np.asarray(inputs["H2frame"], np.float32).reshape(E, 6)], 1)
    hyp_E = np.asarray(inputs["HyperPT"], np.float32).reshape(E, 4)
    th_E = np.ascontiguousarray(ea[:, 9])
    om_E = np.asarray(inputs["option_mask"]).astype(np.float32)
    x = np.asarray(inputs["x"], np.float32)
    att = np.asarray(inputs["att"], np.float32)
    cv = np.asarray(inputs["c"], np.float32)

    try:
        _build_programs()
        if "RG" not in _CACHE:
            _CACHE["RG"] = _Runner(_CACHE["G"])
            _CACHE["RS"] = _Runner(_CACHE["S"])

        xpad4 = np.zeros((NC * R, 4), np.float32)
        xpad4[:N, :3] = x
        attB = np.tile(att[0, 3:6], (P, 1)).astype(np.float32)
        gcores = _prep_pass(src, NTILE, NWIN)
        gpos_core = np.zeros(BE, np.int32)
        gpos_slot = np.zeros(BE, np.int64)
        gmaps = []
        for c, info in enumerate(gcores):
            idx, slot = info["idx"], info["slot"]
            gpos_core[idx] = c
            gpos_slot[idx] = slot
            S_ = NTILE * P
            er = erow[idx]
            ev18 = np.zeros((S_, 18), np.float32)
            ev18[slot, :15] = ev15_E[er]
            hyp = np.zeros((S_, 4), np.float32)
            hyp[slot] = hyp_E[er]
            th = np.zeros(S_, np.float32)
            th[slot] = th_E[er]
            om = np.full(S_, 9.0, np.float32)
            om[slot] = om_E[er]
            xg = xpad4[c * R:(c + 1) * R].reshape(NWIN, P, 4).transpose(1, 0, 2)
            gmaps.append({
                "xg": np.ascontiguousarray(xg.reshape(P, NWIN * 4)),
                "ev18": _slot_layout(ev18, NTILE, 18),
                "hyp": _slot_layout(hyp, NTILE, 4),
                "th": _slot_layout(th, NTILE),
                "om": _slot_layout(om, NTILE),
                "srclf": info["keyl"].reshape(1, NTILE * P),
                "wt4": (info["wt"] * 4).astype(np.int32).reshape(1, NTILE),
                "attB": attB,
            })
        g_res, tg = _CACHE["RG"].run(gmaps)

        ptu_edges = np.zeros((BE, 4), np.float32)
        for c in range(NC):
            m = gpos_core == c
            g = g_res[c]["ptu"]
            sl = gpos_slot[m]
            ptu_edges[m] = g[sl % P, sl // P]

        xpad3 = np.zeros((NC * R, 3), np.float32)
        xpad3[:N] = x
        attA = np.tile(att[0, 0:3], (P, 1)).astype(np.float32)
        kc = np.full((P, 1), float(cv[0] + cv[1]), np.float32)
        iotaP = np.tile(np.arange(P, dtype=np.float32), (P, 1))
        scores_ = _prep_pass(dst, NTILE, NWIN)
        smaps = []
        for c, info in enumerate(scores_):
            idx, slot = info["idx"], info["slot"]
            S_ = NTILE * P
            ptu = np.zeros((S_, 4), np.float32)
            ptu[:, 3] = -1e5
            ptu[slot] = ptu_edges[idx]
            wws = np.empty(NTILE * 2, np.int32)
            wws[0::2] = info["wt"]
            wws[1::2] = info["wt"] * 4
            xt = xpad3[c * R:(c + 1) * R].reshape(NWIN, P, 3).transpose(1, 0, 2)
            smaps.append({
                "xt": np.ascontiguousarray(xt),
                "ptu": _slot_layout(ptu, NTILE, 4),
                "dstl": _slot_layout(info["keyl"], NTILE),
                "dstlf": info["keyl"].reshape(1, NTILE * P),
                "wws": wws.reshape(1, NTILE * 2),
                "attA": attA, "kc": kc, "iotaP": iotaP,
            })
        s_res, ts = _CACHE["RS"].run(smaps)
        _CACHE["last_times"] = (tg, ts)
        out = np.concatenate(
            [s_res[c]["outw"].transpose(1, 0, 2).reshape(R, 3) for c in range(NC)],
            axis=0)[:N]
        return np.ascontiguousarray(out)
    except Exception as exc:  # out-of-envelope inputs: stay correct
        print(f"kernel: device path failed ({exc!r}); numpy fallback", file=sys.stderr)
        return _numpy_fallback(inputs)



# revision 2
# speedup vs baseline: 1.2545x; 1.2545x over previous
"""CURVGT GNN message-passing kernel for 8 TRN2 NeuronCores — single fused
device program (one jit dispatch).

Edges are sharded by dst node range (37504 nodes/core) and sorted by dst into
128-slot tiles aligned to 128-node dst windows (per-window tile capacity is
computed from the input's window histogram, shared across cores). Per chunk
the program:
  - gathers x_j rows on device via per-tile GPSIMD indirect DMA (int32 global
    src offsets into the replicated padded x table),
  - computes parallel transport pt and u = <pt, att[3:6]> per edge,
  - computes g_i = <x_i, att[0:3]> per edge with a per-node g2 table (built on
    device from the core's window-transposed x slice) expanded to slots via a
    one-hot matmul,
  - accumulates segment-softmax numerator/denominator into a PSUM-resident
    per-node table via static one-hot matmuls, finalized as num/(den+1e-16).
Host work is limited to sharding/layout (sorting edge ids, slot streams,
output unscramble). Exploits k=k2=ones, attn_p=ones (verified at runtime):
the curvature branch reduces to m1=m2=sum(pt)*ones, feats=0, s=0, lin=b1
(constant shift, cancels in softmax), as in the spec's input distribution.
"""
import sys, math, time
sys.path.insert(0, "/opt/trn_rl_repo")
import numpy as np

P = 128
V, E, B = 150000, 900000, 2
N = B * V
BE = B * E
NC = 8
RV = 37504              # nodes per core slice (8*37504 = 300032 >= N)
NWIN = RV // P          # 293 dst windows per core
XROWS = 300040          # padded x table rows; row 300032 is the pad target
CT = 32                 # tiles per chunk

_CACHE = {}


def _build_F(caps, starts, stops):
    """caps: per-window tile counts (len 293); starts/stops: per-tile flags."""
    import concourse.bacc as bacc
    import concourse.bass as bass
    import concourse.mybir as mybir
    import concourse.tile as tile

    F = mybir.dt.float32
    BF = mybir.dt.bfloat16
    I32 = mybir.dt.int32
    AF = mybir.ActivationFunctionType
    ALU = mybir.AluOpType
    AX = mybir.AxisListType

    NT = int(np.sum(caps))
    assert NT % CT == 0
    NCH = NT // CT
    wins = np.repeat(np.arange(NWIN), caps)          # tile -> window

    nc = bacc.Bacc("TRN2", target_bir_lowering=False, debug=False,
                   num_devices=NC)
    xf_d = nc.dram_tensor("xf", [XROWS, 16], F, kind="ExternalInput").ap()
    xw_d = nc.dram_tensor("xwin", [P, NWIN, 16], F, kind="ExternalInput").ap()
    ev_d = nc.dram_tensor("ev24", [P, NT, 24], F, kind="ExternalInput").ap()
    dl_d = nc.dram_tensor("dloc", [P, NT], F, kind="ExternalInput").ap()
    dlf_d = nc.dram_tensor("dlocf", [1, NT * P], F, kind="ExternalInput").ap()
    so_d = nc.dram_tensor("soff", [P, NT], I32, kind="ExternalInput").ap()
    iota_d = nc.dram_tensor("iotaP", [P, P], F, kind="ExternalInput").ap()
    attA_d = nc.dram_tensor("attA", [P, 3], F, kind="ExternalInput").ap()
    attB_d = nc.dram_tensor("attB", [P, 3], F, kind="ExternalInput").ap()
    kc_d = nc.dram_tensor("kc", [P, 1], F, kind="ExternalInput").ap()
    out_d = nc.dram_tensor("outw", [P, NWIN, 3], F, kind="ExternalOutput").ap()

    with tile.TileContext(nc) as tc:
        with tc.tile_pool(name="cst", bufs=1) as cst, \
             tc.tile_pool(name="sb", bufs=2) as sb, \
             tc.tile_pool(name="psa", bufs=1, space="PSUM") as psa, \
             tc.tile_pool(name="psg", bufs=2, space="PSUM") as psg:
            iotaP = cst.tile([P, P], F)
            nc.sync.dma_start(out=iotaP[:], in_=iota_d[:])
            attA = cst.tile([P, 3], F)
            nc.sync.dma_start(out=attA[:], in_=attA_d[:])
            attB = cst.tile([P, 3], F)
            nc.sync.dma_start(out=attB[:], in_=attB_d[:])
            kc = cst.tile([P, 1], F)
            nc.sync.dma_start(out=kc[:], in_=kc_d[:])
            soff = cst.tile([P, NT], I32)
            nc.sync.dma_start(out=soff[:], in_=so_d[:])
            dloc = cst.tile([P, NT], F)
            nc.sync.dma_start(out=dloc[:], in_=dl_d[:])
            iop_i = cst.tile([P, 1], I32)
            nc.gpsimd.iota(iop_i[:], pattern=[[0, 1]], base=0,
                           channel_multiplier=1)
            iop = cst.tile([P, 1], F)
            nc.vector.tensor_copy(out=iop[:], in_=iop_i[:])

            # per-node g2 = <x_i, att[0:3]> for this core's dst slice
            xwin = cst.tile([P, NWIN, 16], F)
            nc.sync.dma_start(out=xwin[:], in_=xw_d[:])
            g2m = cst.tile([P, NWIN, 3], F)
            nc.vector.tensor_tensor(
                out=g2m[:], in0=xwin[:, :, 0:3],
                in1=attA[:].rearrange("p (o c) -> p o c", o=1)
                    .to_broadcast([P, NWIN, 3]),
                op=ALU.mult)
            g2f = cst.tile([P, NWIN], F)
            nc.vector.tensor_reduce(out=g2f[:], in_=g2m[:], axis=AX.X,
                                    op=ALU.add)
            g2sb = cst.tile([P, NWIN], BF)
            nc.vector.tensor_copy(out=g2sb[:], in_=g2f[:])

            acc = psa.tile([P, NWIN * 4], F)

            for ch in range(NCH):
                t0 = ch * CT
                ne = CT * P
                xjb = sb.tile([P, CT, 16], F, tag="xjb")
                for j in range(CT):
                    nc.gpsimd.indirect_dma_start(
                        out=xjb[:, j], out_offset=None, in_=xf_d[:],
                        in_offset=bass.IndirectOffsetOnAxis(
                            ap=soff[:, t0 + j:t0 + j + 1], axis=0))
                ev = sb.tile([P, CT, 24], F, tag="ev")
                nc.sync.dma_start(out=ev[:], in_=ev_d[:, t0:t0 + CT])
                nc.vector.tensor_copy(out=ev[:, :, 15:18], in_=xjb[:, :, 0:3])
                th = ev[:, :, 18]

                # cos/sin with one-period range reduction (|theta| < 3*pi)
                cs = sb.tile([P, CT, 2], F, tag="cs")
                g1 = sb.tile([P, CT], F, tag="g1")
                g2_ = sb.tile([P, CT], F, tag="g2_")
                d2 = sb.tile([P, CT], F, tag="d2")
                thr = sb.tile([P, CT], F, tag="thr")
                nc.vector.tensor_scalar(g1[:], th, math.pi, None, ALU.is_gt)
                nc.vector.tensor_scalar(g2_[:], th, -math.pi, None, ALU.is_lt)
                nc.vector.tensor_tensor(out=d2[:], in0=g1[:], in1=g2_[:],
                                        op=ALU.subtract)
                nc.vector.tensor_scalar(d2[:], d2[:], 2 * math.pi, None,
                                        ALU.mult)
                nc.vector.tensor_tensor(out=thr[:], in0=th, in1=d2[:],
                                        op=ALU.subtract)
                nc.scalar.activation(cs[:, :, 1], thr[:], AF.Sin)
                thc = sb.tile([P, CT], F, tag="thc")
                nc.vector.tensor_scalar(thc[:], th, math.pi / 2, None, ALU.add)
                nc.vector.tensor_scalar(g1[:], thc[:], math.pi, None, ALU.is_gt)
                nc.vector.tensor_scalar(g2_[:], thc[:], -math.pi, None,
                                        ALU.is_lt)
                nc.vector.tensor_tensor(out=d2[:], in0=g1[:], in1=g2_[:],
                                        op=ALU.subtract)
                nc.vector.tensor_scalar(d2[:], d2[:], 2 * math.pi, None,
                                        ALU.mult)
                nc.vector.tensor_tensor(out=thc[:], in0=thc[:], in1=d2[:],
                                        op=ALU.subtract)
                nc.scalar.activation(cs[:, :, 0], thc[:], AF.Sin)

                t6 = sb.tile([P, CT, 2, 3], F, tag="t6")
                ab = sb.tile([P, CT, 2], F, tag="ab")
                nc.vector.tensor_tensor(
                    out=t6[:],
                    in0=ev[:, :, 0:6].rearrange("p t (v c) -> p t v c", c=3),
                    in1=xjb[:, :, 0:3].rearrange("p t (o c) -> p t o c", o=1)
                        .to_broadcast([P, CT, 2, 3]),
                    op=ALU.mult)
                nc.vector.tensor_reduce(out=ab[:], in_=t6[:], axis=AX.X,
                                        op=ALU.add)
                t6b = sb.tile([P, CT, 2, 3], F, tag="t6b")
                ab2 = sb.tile([P, CT, 2], F, tag="ab2")
                nc.vector.tensor_tensor(
                    out=t6b[:],
                    in0=ev[:, :, 9:15].rearrange("p t (v c) -> p t v c", c=3),
                    in1=xjb[:, :, 0:3].rearrange("p t (o c) -> p t o c", o=1)
                        .to_broadcast([P, CT, 2, 3]),
                    op=ALU.mult)
                nc.vector.tensor_reduce(out=ab2[:], in_=t6b[:], axis=AX.X,
                                        op=ALU.add)
                t4 = sb.tile([P, CT, 2, 2], F, tag="t4")
                lc = sb.tile([P, CT, 2], F, tag="lc")
                nc.vector.tensor_tensor(
                    out=t4[:],
                    in0=ev[:, :, 19:23].rearrange("p t (v c) -> p t v c", c=2),
                    in1=ab2[:].rearrange("p t (o c) -> p t o c", o=1)
                        .to_broadcast([P, CT, 2, 2]),
                    op=ALU.mult)
                nc.vector.tensor_reduce(out=lc[:], in_=t4[:], axis=AX.X,
                                        op=ALU.add)

                m1 = sb.tile([P, CT], F, tag="m1")
                nc.vector.tensor_scalar(m1[:], ev[:, :, 23], 1.0, None,
                                        ALU.is_equal)
                mm = sb.tile([P, CT], F, tag="mm")
                nc.vector.tensor_scalar(mm[:], ev[:, :, 23], -1.0, None,
                                        ALU.is_equal)
                m0 = sb.tile([P, CT], F, tag="m0")
                nc.vector.tensor_scalar(m0[:], ev[:, :, 23], 0.0, None,
                                        ALU.is_equal)

                co = sb.tile([P, CT, 6], F, tag="co")
                am1 = sb.tile([P, CT], F, tag="am1")
                nc.vector.tensor_tensor(out=am1[:], in0=ab[:, :, 0],
                                        in1=m1[:], op=ALU.mult)
                nc.vector.tensor_tensor(
                    out=co[:, :, 0:3:2],
                    in0=am1[:].rearrange("p (t o) -> p t o", o=1)
                        .to_broadcast([P, CT, 2]),
                    in1=cs[:], op=ALU.mult)
                nc.vector.tensor_tensor(out=co[:, :, 1], in0=ab[:, :, 1],
                                        in1=m1[:], op=ALU.mult)
                nc.vector.tensor_tensor(
                    out=co[:, :, 3:5], in0=lc[:],
                    in1=mm[:].rearrange("p (t o) -> p t o", o=1)
                        .to_broadcast([P, CT, 2]),
                    op=ALU.mult)
                nc.vector.tensor_copy(out=co[:, :, 5], in_=m0[:])

                big = sb.tile([P, CT, 3, 6], F, tag="big")
                ptu = sb.tile([P, CT, 4], F, tag="ptu")
                nc.vector.tensor_tensor(
                    out=big[:],
                    in0=co[:].rearrange("p t (o k) -> p t o k", o=1)
                        .to_broadcast([P, CT, 3, 6]),
                    in1=ev[:, :, 0:18].rearrange("p t (k c) -> p t c k", c=3),
                    op=ALU.mult)
                nc.vector.tensor_reduce(out=ptu[:, :, 0:3], in_=big[:],
                                        axis=AX.X, op=ALU.add)
                t3 = sb.tile([P, CT, 3], F, tag="t3")
                nc.vector.tensor_tensor(
                    out=t3[:], in0=ptu[:, :, 0:3],
                    in1=attB[:].rearrange("p (o c) -> p o c", o=1)
                        .to_broadcast([P, CT, 3]),
                    op=ALU.mult)
                nc.vector.tensor_reduce(out=ptu[:, :, 3], in_=t3[:],
                                        axis=AX.X, op=ALU.add)

                # one-hots: ohS (partition=slot, free=local) from dloc stream;
                # oh2 (partition=local, free=slot) from flat stream broadcast
                ohS = sb.tile([P, CT, P], BF, tag="ohS")
                nc.vector.tensor_tensor(
                    out=ohS[:],
                    in0=iotaP[:].rearrange("p (o k) -> p o k", o=1)
                        .to_broadcast([P, CT, P]),
                    in1=dloc[:, t0:t0 + CT]
                        .rearrange("p (t o) -> p t o", o=1)
                        .to_broadcast([P, CT, P]),
                    op=ALU.is_equal)
                dlf = sb.tile([1, CT * P], F, tag="dlf")
                nc.sync.dma_start(out=dlf[:], in_=dlf_d[:, t0 * P:t0 * P + ne])
                dlr = sb.tile([P, CT * P], F, tag="dlr")
                nc.gpsimd.partition_broadcast(dlr[:], dlf[:1, :])
                oh2 = sb.tile([P, CT, P], BF, tag="oh2")
                nc.vector.tensor_tensor(
                    out=oh2[:].rearrange("p t s -> p (t s)"),
                    in0=iop[:].to_broadcast([P, ne]),
                    in1=dlr[:], op=ALU.is_equal)

                # g_i per slot via one-hot matmul against g2 column
                gi = sb.tile([P, CT], F, tag="gi")
                for j in range(CT):
                    w = int(wins[t0 + j])
                    gps = psg.tile([P, 1], F, tag="gps")
                    nc.tensor.matmul(out=gps[:], lhsT=oh2[:, j],
                                     rhs=g2sb[:, w:w + 1], start=True,
                                     stop=True)
                    nc.vector.tensor_copy(out=gi[:, j:j + 1], in_=gps[:])

                z = sb.tile([P, CT], F, tag="z")
                nc.vector.tensor_tensor(out=z[:], in0=gi[:], in1=ptu[:, :, 3],
                                        op=ALU.add)
                z2 = sb.tile([P, CT], F, tag="z2")
                nc.vector.tensor_scalar(z2[:], z[:], 0.2, None, ALU.mult)
                gat = sb.tile([P, CT], F, tag="gat")
                nc.vector.tensor_tensor(out=gat[:], in0=z[:], in1=z2[:],
                                        op=ALU.max)
                ex = sb.tile([P, CT], F, tag="ex")
                nc.scalar.activation(ex[:], gat[:], AF.Exp)
                s = sb.tile([P, CT], F, tag="s")
                nc.vector.tensor_reduce(out=s[:], in_=ptu[:, :, 0:3],
                                        axis=AX.X, op=ALU.add)
                ks = sb.tile([P, CT], F, tag="ks")
                nc.vector.tensor_scalar(ks[:], s[:], kc[:, 0:1], None,
                                        ALU.mult)
                v3 = sb.tile([P, CT, 3], F, tag="v3")
                nc.vector.tensor_tensor(
                    out=v3[:], in0=ptu[:, :, 0:3],
                    in1=ks[:].rearrange("p (t o) -> p t o", o=1)
                        .to_broadcast([P, CT, 3]),
                    op=ALU.add)
                pay = sb.tile([P, CT, 4], BF, tag="pay")
                nc.vector.tensor_copy(out=pay[:, :, 0], in_=ex[:])
                nc.vector.tensor_tensor(
                    out=pay[:, :, 1:4], in0=v3[:],
                    in1=ex[:].rearrange("p (t o) -> p t o", o=1)
                        .to_broadcast([P, CT, 3]),
                    op=ALU.mult)

                for j in range(CT):
                    t = t0 + j
                    w = int(wins[t])
                    nc.tensor.matmul(
                        out=acc[:, 4 * w:4 * w + 4], lhsT=ohS[:, j],
                        rhs=pay[:, j], start=bool(starts[t]),
                        stop=bool(stops[t]))

            accs = cst.tile([P, NWIN, 4], F)
            nc.vector.tensor_copy(
                out=accs[:], in_=acc[:].rearrange("p (w c) -> p w c", c=4))
            den = cst.tile([P, NWIN], F)
            nc.vector.tensor_scalar(den[:], accs[:, :, 0], 1e-16, None,
                                    ALU.add)
            rec = cst.tile([P, NWIN], F)
            nc.vector.reciprocal(rec[:], den[:])
            outw = cst.tile([P, NWIN, 3], F)
            nc.vector.tensor_tensor(
                out=outw[:], in0=accs[:, :, 1:4],
                in1=rec[:].rearrange("p (w o) -> p w o", o=1)
                    .to_broadcast([P, NWIN, 3]),
                op=ALU.mult)
            nc.sync.dma_start(out=out_d[:], in_=outw[:])
    nc.compile()
    return nc


class _Runner:
    def __init__(self, nc):
        import jax
        import jax.numpy  # noqa
        from jax.sharding import Mesh, PartitionSpec, NamedSharding
        from jax.experimental.shard_map import shard_map
        import concourse.mybir as mybir
        from concourse.bass2jax import (_bass_exec_p, install_neuronx_cc_hook,
                                        partition_id_tensor)
        install_neuronx_cc_hook()
        self.jax = jax
        in_names, out_names, out_avals, zero_outs = [], [], [], []
        pname = nc.partition_id_tensor.name if nc.partition_id_tensor else None
        for alloc in nc.m.functions[0].allocations:
            if not isinstance(alloc, mybir.MemoryLocationSet):
                continue
            name = alloc.memorylocations[0].name
            if alloc.kind == "ExternalInput":
                if name != pname:
                    in_names.append(name)
            elif alloc.kind == "ExternalOutput":
                shape = tuple(alloc.tensor_shape)
                dtype = mybir.dt.np(alloc.dtype)
                out_names.append(name)
                out_avals.append(jax.core.ShapedArray(shape, dtype))
                zero_outs.append(np.zeros(shape, dtype))
        self.in_names, self.out_names, self.zero_outs = in_names, out_names, zero_outs
        n_params, n_outs = len(in_names), len(out_names)
        all_names = list(in_names) + list(out_names)
        if pname is not None:
            all_names.append(pname)

        def _body(*args):
            operands = list(args)
            if pname is not None:
                operands.append(partition_id_tensor())
            return tuple(_bass_exec_p.bind(
                *operands, out_avals=tuple(out_avals), in_names=tuple(all_names),
                out_names=tuple(out_names), lowering_input_output_aliases=(),
                sim_require_finite=False, sim_require_nnan=False, nc=nc))

        devices = jax.devices()[:NC]
        mesh = Mesh(np.asarray(devices), ("core",))
        in_specs = (PartitionSpec("core"),) * (n_params + n_outs)
        out_specs = (PartitionSpec("core"),) * n_outs
        self.fn = jax.jit(
            shard_map(_body, mesh=mesh, in_specs=in_specs, out_specs=out_specs,
                      check_rep=False),
            donate_argnums=tuple(range(n_params, n_params + n_outs)),
            keep_unused=True)
        self.sharding = NamedSharding(mesh, PartitionSpec("core"))

    def run(self, in_maps):
        jax = self.jax
        dev_in = [jax.device_put(
            np.concatenate([np.asarray(m[n]) for m in in_maps], axis=0),
            self.sharding) for n in self.in_names]
        dev_out = [jax.device_put(np.concatenate([z] * NC, axis=0), self.sharding)
                   for z in self.zero_outs]
        jax.block_until_ready(dev_in)
        jax.block_until_ready(dev_out)
        t0 = time.perf_counter()
        outs = self.fn(*dev_in, *dev_out)
        jax.block_until_ready(outs)
        dt = time.perf_counter() - t0
        res = [dict() for _ in range(NC)]
        for name, arr in zip(self.out_names, outs):
            arr = np.asarray(arr)
            per = arr.shape[0] // NC
            for c in range(NC):
                res[c][name] = arr[c * per:(c + 1) * per]
        return res, dt


def _slot_layout(arr_slots, ntile, k=None):
    if k is None:
        return np.ascontiguousarray(arr_slots.reshape(ntile, P).T)
    return np.ascontiguousarray(arr_slots.reshape(ntile, P, k).transpose(1, 0, 2))


def _numpy_fallback(inputs):
    def _ln(x, axes):
        mu = x.mean(axis=axes, keepdims=True)
        var = x.var(axis=axes, keepdims=True)
        return (x - mu) / np.sqrt(var + 1e-5)

    x = np.asarray(inputs["x"], np.float32)
    ei = np.asarray(inputs["edge_index"]).astype(np.int64)
    ea = np.asarray(inputs["edge_attrs"], np.float32)
    H2 = np.asarray(inputs["H2frame"], np.float32)
    HPT = np.asarray(inputs["HyperPT"], np.float32)
    omi = np.asarray(inputs["option_mask"]).astype(np.int64)
    bm = np.asarray(inputs["broadcastmap"]).astype(np.int64)
    k = np.asarray(inputs["k"], np.float32); k2 = np.asarray(inputs["k2"], np.float32)
    ap_ = np.asarray(inputs["attn_p"], np.float32)
    att = np.asarray(inputs["att"], np.float32)
    W1 = np.asarray(inputs["W1"], np.float32); b1 = np.asarray(inputs["b1"], np.float32)
    cv = np.asarray(inputs["c"], np.float32)
    src, dst = ei[0], ei[1]

    def tile(a):
        return np.tile(a, (B,) + (1,) * (a.ndim - 1))

    Theta = tile(ea[:, 9:10]); e1 = tile(ea[:, 11:14]); e2 = tile(ea[:, 14:17])
    e3 = tile(ea[:, 17:20]); cos, sin = np.cos(Theta), np.sin(Theta)
    xdir, ydir = tile(H2[:, 0]), tile(H2[:, 1]); T = tile(HPT)
    om = np.tile(omi, B)
    x_j = x[src]; x_i = x[dst]
    a = (e1 * x_j).sum(-1, keepdims=True)
    b = (e2 * x_j).sum(-1, keepdims=True)
    pt1 = a * cos * e1 + a * sin * e3 + b * e2
    a2 = (xdir * x_j).sum(-1, keepdims=True)
    b2 = (ydir * x_j).sum(-1, keepdims=True)
    local = np.concatenate([a2, b2], -1)
    lc2 = np.einsum("eij,ej->ei", T, local)
    pt2 = xdir * lc2[:, 0:1] + ydir * lc2[:, 1:2]
    pt = (pt1 * (om == 1)[:, None] + pt2 * (om == -1)[:, None]
          + x_j * (om == 0)[:, None])
    roots = bm[dst % V]
    m1 = np.einsum("eij,ej->ei", k[roots], pt)
    m2 = np.einsum("eij,ej->ei", k2[roots], pt)
    feats = _ln(np.stack([m1, m2], -1), (1, 2))
    sv = _ln(np.einsum("ecd,edc->ec", ap_[roots], feats), (1,))
    z = np.concatenate([x_i, pt], -1) @ att[0]
    gat = np.where(z > 0, z, 0.2 * z)
    lin = (sv @ W1.T + b1)[:, 0]
    score = gat + lin
    smax = np.full(N, -np.inf, np.float32)
    np.maximum.at(smax, dst, score)
    exps = np.exp(score - smax[dst])
    denom = np.zeros(N, np.float32)
    np.add.at(denom, dst, exps)
    alpha = exps / (denom[dst] + 1e-16)
    msg = alpha[:, None] * (pt + cv[0] * m1 + cv[1] * m2)
    out = np.zeros((N, 3), np.float32)
    np.add.at(out, dst, msg)
    return out


def _device_kernel(inputs):
    ei = np.asarray(inputs["edge_index"]).astype(np.int64)
    src, dst = ei[0], ei[1]
    erow = np.arange(BE) % E
    ea = np.asarray(inputs["edge_attrs"], np.float32)
    ev15_E = np.concatenate(
        [ea[:, 11:20], np.asarray(inputs["H2frame"], np.float32).reshape(E, 6)], 1)
    hyp_E = np.asarray(inputs["HyperPT"], np.float32).reshape(E, 4)
    th_E = np.ascontiguousarray(ea[:, 9])
    om_E = np.asarray(inputs["option_mask"]).astype(np.float32)
    x = np.asarray(inputs["x"], np.float32)
    att = np.asarray(inputs["att"], np.float32)
    cv = np.asarray(inputs["c"], np.float32)

    xf = np.zeros((XROWS, 16), np.float32)
    xf[:N, :3] = x
    attA = np.tile(att[0, 0:3], (P, 1)).astype(np.float32)
    attB = np.tile(att[0, 3:6], (P, 1)).astype(np.float32)
    kc = np.full((P, 1), float(cv[0] + cv[1]), np.float32)
    iotaP = np.tile(np.arange(P, dtype=np.float32), (P, 1))

    order = np.argsort(dst, kind="stable")
    bounds = np.searchsorted(dst[order], np.arange(NC + 1) * RV)
    # per-(core, window) counts -> shared tile capacities
    cnts = np.zeros((NC, NWIN), np.int64)
    percore = []
    for c in range(NC):
        idx = order[bounds[c]:bounds[c + 1]]
        dl = (dst[idx] - c * RV).astype(np.int64)
        w = dl >> 7
        cnts[c] = np.bincount(w, minlength=NWIN)
        percore.append((idx, dl, w))
    caps = np.maximum((cnts.max(axis=0) + P - 1) // P, 1).astype(np.int64)
    pad = (-int(caps.sum())) % CT
    for i in range(pad):
        caps[NWIN - 1 - i] += 1
    NT = int(caps.sum())
    if NT > 4096:
        raise RuntimeError(f"tile count {NT} too large")
    tile_base = np.concatenate([[0], np.cumsum(caps)]).astype(np.int64)
    starts = np.zeros(NT, bool)
    stops = np.zeros(NT, bool)
    starts[tile_base[:-1]] = True
    stops[tile_base[1:] - 1] = True

    key = ("F", tuple(int(v) for v in caps))
    if key not in _CACHE:
        _CACHE[key] = _build_F(caps, starts, stops)
        _CACHE[("R", key)] = _Runner(_CACHE[key])
    runner = _CACHE[("R", key)]

    SS = NT * P
    in_maps = []
    for c in range(NC):
        idx, dl, w = percore[c]
        gstart = np.concatenate([[0], np.cumsum(cnts[c])]).astype(np.int64)
        jj = np.arange(len(idx)) - gstart[w]
        slot = (tile_base[w] + (jj >> 7)) * P + (jj & 127)
        er = erow[idx]
        ev = np.zeros((SS, 24), np.float32)
        ev[slot, 0:15] = ev15_E[er]
        ev[slot, 18] = th_E[er]
        ev[slot, 19:23] = hyp_E[er]
        ev[slot, 23] = om_E[er]
        mask = np.ones(SS, bool)
        mask[slot] = False
        ev[mask, 23] = 9.0
        dloc_arr = np.full(SS, 999.0, np.float32)
        dloc_arr[slot] = (dl & 127).astype(np.float32)
        soff_arr = np.full(SS, N + 32, np.int64)  # pad -> zero row 300032
        soff_arr[slot] = src[idx]
        xwin = np.zeros((NWIN * P, 16), np.float32)
        hi = min((c + 1) * RV, N)
        xwin[:hi - c * RV, :3] = x[c * RV:hi]
        in_maps.append({
            "xf": xf,
            "xwin": np.ascontiguousarray(
                xwin.reshape(NWIN, P, 16).transpose(1, 0, 2)),
            "ev24": _slot_layout(ev, NT, 24),
            "dloc": _slot_layout(dloc_arr, NT),
            "dlocf": dloc_arr.reshape(1, SS),
            "soff": _slot_layout(soff_arr.astype(np.int32), NT),
            "iotaP": iotaP, "attA": attA, "attB": attB, "kc": kc,
        })
    res, tf = runner.run(in_maps)
    _CACHE["last_times"] = (tf,)
    out = np.concatenate(
        [res[c]["outw"].transpose(1, 0, 2).reshape(RV, 3) for c in range(NC)],
        axis=0)[:N]
    return np.ascontiguousarray(out)


def kernel(**inputs):
    # simplification requires ones-filled curvature tensors (per spec fill)
    ok = (np.all(np.asarray(inputs["k"]) == 1.0)
          and np.all(np.asarray(inputs["k2"]) == 1.0)
          and np.all(np.asarray(inputs["attn_p"]) == 1.0))
    if not ok:
        return _numpy_fallback(inputs)
    try:
        return _device_kernel(inputs)
    except Exception as exc:  # out-of-envelope inputs: stay correct
        print(f"kernel: device path failed ({exc!r}); numpy fallback",
              file=sys.stderr)
        return _numpy_fallback(inputs)
